# revision 21
# baseline (speedup 1.0000x reference)
"""Trainium2 Bass kernel for nn_ChannelAttention (squeeze-excite).

Reference computation:
    s = mean(x, axis=(H, W))                    # [B, C]   global avg pool
    h = relu(bn1(s @ w1))                       # [B, Cr]  Cr = 16
    o = bn2(h @ w2)                             # [B, C]
    return o[:, None, None, :]                  # [B, 1, 1, C]

Strategy (data-parallel over batch, 8 cores x 8 samples). The kernel is
HBM-stream-bound: 25.7 MB of x per core, moved by 16 HW DMA queues that
each sustain ~27.1 GB/s (descriptor = one SBUF partition row, assigned
round-robin row % 16). v2 changes vs the 97-100 us baseline:

  - Each sample-PAIR tile [128, 12544] is DMA'd as 4-5 COLUMN-CHUNK
    dma_starts (~3072 cols each) instead of one 6.4 MB transfer, so
    consumers wake per-chunk: compute starts at ~12 us (not ~29) and
    engine backlog never piles up at the stream tail. The final pair
    ends with a deliberately tiny 768-col chunk so the post-last-byte
    critical chain starts with ~1.3 us of PE work, not a whole chunk.
  - BatchNorm is folded on the HOST into the packed parameters
    (standard inference-time constant folding): w1k = w1*k1/HW,
    b1 = beta1-mean1*k1, w2k = w2*k2, b2 = beta2-mean2*k2 with
    k = gamma/sqrt(var+eps) computed in float64. The device runs NO
    BN math - DVE does nothing but stream pre-reduction adds.
  - Squeeze: per 512-col slice, PE reduces with an M=33 pair-indicator
    lhsT (sample sums land on 32-aligned PSUM rows {0,32}) or DVE
    pre-reduces a chain into a 512-wide accumulator that PE folds in.
    Split tuned so PE ~63% / DVE ~70% busy during the stream.
  - The [1,512]->[1,256] position-half fold is absorbed into the
    GATHER matmuls (lhsT reads each 128-col quarter of the raw PSUM
    copy; one-hot rhs zeroes the garbage rows), removing the per-pair
    DVE fold. PSUM->SBUF copies are split Scalar/GpSimd in parallel.
  - Excite MLP: g1[16,8] = w1k.T @ sT (K=256 in 2 matmuls), one Relu
    activation adds b1, o[8,256] = h_ext.T @ w2bi where w2bi carries
    w2k on rows 0..15 and b2 on row 32 (h_ext row 32 = ones).
  - All 16 memsets/constants live on GpSimd; Scalar only copies + the
    final Relu; activation tables preload during the DMA ramp.
"""

import sys

if "/opt/trn_rl_repo" not in sys.path:
    sys.path.insert(0, "/opt/trn_rl_repo")

import numpy as np

B, H, W, C = 64, 56, 56, 256
CR = 16
NCORES = 8
BL = B // NCORES  # samples per core
HWP = H * W  # 3136 spatial positions
NPAIR = BL // 2  # 4 sample-pairs per core
PFD = 2 * HWP * C // 128  # 12544 free-dim elements per partition
PW = 292  # packed parameter tensor width (see _pack_params)
EPS = 1e-3

# column-chunk boundaries per pair. Short chunks keep DVE chains <= 2
# adds (fat chunks were tried: their 7-add serial chains delayed folds,
# PE queued in-order behind them and the whole tail cascaded ~12us
# late). The final pair tapers so the post-last-byte chain is one
# 256-col matmul.
CHUNKS = [0, 3072, 6144, 9216, PFD]
CHUNKS_LAST = [0, 3584, 7168, 9728, 11264, 12288, PFD]

_CACHE: dict = {}


def _build_nc():
    import concourse.bass as bass
    import concourse.tile as tile
    from concourse import bacc, mybir
    from contextlib import ExitStack

    f32 = mybir.dt.float32
    bf16 = mybir.dt.bfloat16
    AF = mybir.ActivationFunctionType

    nc = bacc.Bacc("TRN2", target_bir_lowering=False, debug=False)

    x_d = nc.dram_tensor("x", [NPAIR, 128, PFD], f32, kind="ExternalInput")
    par_d = nc.dram_tensor("params", [128, PW], f32, kind="ExternalInput")
    out_d = nc.dram_tensor("out", [BL, C], f32, kind="ExternalOutput")

    with ExitStack() as ctx:
        tc = ctx.enter_context(tile.TileContext(nc))
        xp = ctx.enter_context(tc.tile_pool(name="xp", bufs=3))
        pp = ctx.enter_context(tc.tile_pool(name="pp", bufs=1))
        accp = ctx.enter_context(tc.tile_pool(name="accp", bufs=3, space="PSUM"))
        mlpp = ctx.enter_context(tc.tile_pool(name="mlpp", bufs=1, space="PSUM"))

        # ---- params first: single small DMA on the scalar ring ----
        pt = pp.tile([128, PW], f32, tag="pt", name="pt")
        nc.scalar.dma_start(pt, par_d[:, :])
        w1a = pt[:, 0:CR]
        w1b = pt[:, CR : 2 * CR]
        # w2k/b2 are packed host-side as bf16 pairs inside f32 slots;
        # bitcast reinterprets [33, 128] f32 as [33, 256] bf16 so the
        # final matmul runs single-pass (fp32 matmul = 2 half passes)
        w2bi = pt[0:33, 32 : 32 + C // 2].bitcast(bf16)
        b1 = pt[0:CR, 288:289]

        # ---- x stream: column-chunked dma_starts per pair tile so the
        # compute engines wake on ~1.6 MB completions instead of 6.4 MB.
        xts = []
        for q in range(NPAIR):
            xt = xp.tile([128, PFD], f32, tag="xt", name=f"xt{q}", bufs=3)
            cb = CHUNKS_LAST if q == NPAIR - 1 else CHUNKS
            for c0, c1 in zip(cb[:-1], cb[1:]):
                nc.sync.dma_start(xt[:, c0:c1], x_d[q][:, c0:c1])
            xts.append(xt)

        # ---- constants (GpSimd so Vector/Scalar streams stay clean) ----
        # pair indicator, M=33: col 0 selects partitions 0..63 (sample A
        # of the pair -> PSUM row 0), col 32 selects 64..127 (sample B ->
        # PSUM row 32; 32-aligned as compute APs require).
        po = pp.tile([128, 33], f32, tag="po", name="po")
        nc.gpsimd.memset(po, 0.0)
        nc.gpsimd.memset(po[0:64, 0:1], 1.0)
        nc.gpsimd.memset(po[64:128, 32:33], 1.0)

        # gather rhs: oh33[32j, q, b] = 1 iff b == 2q + j; all other rows
        # zero so PSUM garbage rows in the lhsT multiply to exact 0.0
        oh33 = pp.tile([128, NPAIR, BL], bf16, tag="oh33", name="oh33")
        nc.gpsimd.memset(oh33, 0.0)
        for q in range(NPAIR):
            for jj in range(2):
                b = 2 * q + jj
                nc.gpsimd.memset(oh33[32 * jj : 32 * jj + 1, q, b : b + 1], 1.0)

        # h_ext rows 16..31 zero, row 32 ones (selects the b2 bias row of
        # the augmented second-matmul operand w2bi)
        h_ext = pp.tile([33, BL], bf16, tag="h_ext", name="h_ext")
        nc.gpsimd.memset(h_ext, 0.0)
        nc.gpsimd.memset(h_ext[32:33, :], 1.0)

        # route the Relu bias through Scalar so the activation's only
        # cross-engine wait is on the PE matmul result
        b1c = pp.tile([CR, 1], f32, tag="b1c", name="b1c")
        nc.scalar.copy(b1c, b1)

        # ---- stage 1: squeeze (global sum over H*W per sample/channel) ----
        # acc_sb[32j, q, :]: raw [1, 512] channel sums for sample 2q + j
        # (cols 0:256 / 256:512 are the two position-parity halves; the
        # fold happens inside the gather matmuls)
        # bf16: the copies cast on write, halving gather lhsT loads and
        # letting the gather matmuls run single-pass (error budget is
        # huge: tolerance 2e-2, fp32 pipeline measured 2e-7)
        acc_sb = pp.tile([128, NPAIR, 512], bf16, tag="acc_sb", name="acc_sb")
        sT0 = mlpp.tile([128, BL], f32, tag="sT0", name="sT0")
        sT1 = mlpp.tile([128, BL], f32, tag="sT1", name="sT1")

        # PE directs per chunk (rest goes to a DVE chain + one PE fold);
        # tuned so PE ~63%, DVE ~70% of the 14.8us/pair stream budget
        # (fp32 matmul = 2 half-speed passes, ~858ns per 512 slice;
        # float32r was tried but the verifier requires producer-rounded
        # inputs, which costs a full extra pass). The final pair's last
        # chunks are PE-only so no chain latency follows the last byte.
        ND = {0: 2, 1: 1, 2: 1, 3: 1}
        # final pair: PE direct-share shrinks toward the end so PE and
        # DVE both finish with the stream instead of PE piling up ~4us
        # of serial matmuls after the last byte
        ND_LAST = {0: 2, 1: 2, 2: 2, 3: 1, 4: 0, 5: 1}

        ndve = 0
        for q in range(NPAIR):
            xt = xts[q]
            last_pair = q == NPAIR - 1
            cb = CHUNKS_LAST if last_pair else CHUNKS
            nd_map = ND_LAST if last_pair else ND
            acc = accp.tile([128, 512], f32, tag="acc", name=f"acc{q}")
            first = True
            nchunks = len(cb) - 1
            for ci in range(nchunks):
                c0, c1 = cb[ci], cb[ci + 1]
                nfull = (c1 - c0) // 512
                has_tail = (c1 - c0) % 512 != 0  # trailing 256 columns
                nd = min(nd_map[ci], nfull)
                nchain = nfull - nd
                last_chunk = ci == nchunks - 1
                for k in range(nd):
                    nc.tensor.matmul(
                        acc[0:33, :],
                        po,
                        xt[:, c0 + k * 512 : c0 + (k + 1) * 512],
                        start=first,
                        stop=last_chunk
                        and not has_tail
                        and nchain == 0
                        and k == nd - 1,
                    )
                    first = False
                if nchain == 1:
                    nc.tensor.matmul(
                        acc[0:33, :],
                        po,
                        xt[:, c0 + nd * 512 : c0 + (nd + 1) * 512],
                        start=False,
                        stop=last_chunk and not has_tail,
                    )
                elif nchain > 1:
                    dve = pp.tile(
                        [128, 512], f32, tag="dve", name=f"dve{ndve}", bufs=4
                    )
                    ndve += 1
                    s0 = c0 + nd * 512
                    nc.vector.tensor_add(
                        dve, xt[:, s0 : s0 + 512], xt[:, s0 + 512 : s0 + 1024]
                    )
                    for j in range(2, nchain):
                        nc.vector.tensor_add(
                            dve, dve, xt[:, s0 + j * 512 : s0 + (j + 1) * 512]
                        )
                    nc.tensor.matmul(
                        acc[0:33, :],
                        po,
                        dve,
                        start=False,
                        stop=last_chunk and not has_tail,
                    )
                # the 256-col tail is emitted last so the chunk's stop
                # flag always rides the temporally-final matmul
                if has_tail:
                    nc.tensor.matmul(
                        acc[0:33, 0:256],
                        po,
                        xt[:, c0 + nfull * 512 : c1],
                        start=False,
                        stop=last_chunk,
                    )

            # PSUM -> SBUF on two engines in parallel (halves), then the
            # gathers fold + transpose straight from the raw sums
            nc.scalar.copy(acc_sb[0:33, q, 0:256], acc[0:33, 0:256])
            nc.vector.tensor_copy(acc_sb[0:33, q, 256:512], acc[0:33, 256:512])
            for half in range(2):
                for h, sT in enumerate((sT0, sT1)):
                    nc.tensor.matmul(
                        sT[:, 0:BL],
                        acc_sb[0:33, q, 256 * half + 128 * h : 256 * half + 128 * h + 128],
                        oh33[0:33, q, :],
                        start=(q == 0 and half == 0),
                        stop=(q == NPAIR - 1 and half == 1),
                    )

        # ---- stage 2: excite MLP (BN folded host-side) ----
        sT0s = pp.tile([128, BL], f32, tag="sT0s", name="sT0s")
        nc.scalar.copy(sT0s, sT0)
        sT1s = pp.tile([128, BL], f32, tag="sT1s", name="sT1s")
        nc.vector.tensor_copy(sT1s, sT1)

        g1p = mlpp.tile([CR, BL], f32, tag="g1p", name="g1p")
        nc.tensor.matmul(g1p, w1a, sT0s, start=True, stop=False)
        nc.tensor.matmul(g1p, w1b, sT1s, start=False, stop=True)

        nc.scalar.activation(h_ext[0:CR, :], g1p, AF.Relu, bias=b1c)

        o_p = mlpp.tile([BL, C], f32, tag="o_p", name="o_p")
        nc.tensor.matmul(o_p, h_ext[0:33, 0:BL], w2bi, start=True, stop=True)

        # copy + out-DMA both on Scalar: the doorbell follows the copy on
        # the same sequencer, skipping a cross-engine semaphore hop
        ofin = pp.tile([BL, C], f32, tag="ofin", name="ofin")
        nc.scalar.copy(ofin, o_p)
        nc.scalar.dma_start(out_d[:, :], ofin)

    nc.compile()
    return nc


def _get_nc():
    if "nc" not in _CACHE:
        _CACHE["nc"] = _build_nc()
    return _CACHE["nc"]


def _pack_params(inputs):
    """Fold BN into the dense weights host-side (float64 math)."""

    def g(k):
        return np.asarray(inputs[k], dtype=np.float64)

    import ml_dtypes

    k1 = g("gamma1") / np.sqrt(g("var1") + EPS)
    w1k = g("w1") * k1[None, :] * (1.0 / HWP)
    b1 = g("beta1") - g("mean1") * k1
    k2 = g("gamma2") / np.sqrt(g("var2") + EPS)
    w2k = g("w2") * k2[None, :]
    b2 = g("beta2") - g("mean2") * k2

    # w2bi rows 0..15 = w2k, row 32 = b2, stored bf16 and packed as
    # little-endian pairs into f32 slots (device bitcasts back to bf16)
    w2m = np.zeros((33, C), np.float64)
    w2m[0:CR] = w2k
    w2m[32] = b2
    u16 = w2m.astype(ml_dtypes.bfloat16).view(np.uint16)
    packed = u16[:, 0::2].astype(np.uint32) | (u16[:, 1::2].astype(np.uint32) << 16)

    p = np.zeros((128, PW), np.float32)
    p[:, 0:CR] = w1k[0:128]
    p[:, CR : 2 * CR] = w1k[128:256]
    p[0:33, 32 : 32 + C // 2] = packed.view(np.float32)
    p[0:CR, 288] = b1
    return p


def _in_maps(inputs):
    x = np.ascontiguousarray(np.asarray(inputs["x"], dtype=np.float32))
    params = _pack_params(inputs)
    maps = []
    for c in range(NCORES):
        shard = np.ascontiguousarray(x[c * BL : (c + 1) * BL]).reshape(NPAIR, 128, PFD)
        maps.append({"x": shard, "params": params})
    return maps


def _run(inputs, trace=False):
    from concourse.bass_utils import run_bass_kernel_spmd

    nc = _get_nc()
    res = run_bass_kernel_spmd(
        nc, _in_maps(inputs), core_ids=list(range(NCORES)), trace=trace
    )
    out = np.concatenate([res.results[c]["out"] for c in range(NCORES)], axis=0)
    return out.reshape(B, 1, 1, C).astype(np.float32), res


def kernel(**inputs) -> np.ndarray:
    out, _ = _run(inputs, trace=False)
    return out


def kernel_traced(**inputs):
    """Returns (out, BassKernelResults) with NTFF profiling enabled."""
    return _run(inputs, trace=True)


def bench(inputs, iters=30, warmup=5):
    """Time the per-step NEFF execution with device-resident inputs.

    Returns (out_full, per_call_seconds_list). Inputs are device_put once;
    each timed call only dispatches the compiled executable, so steady-state
    per-call wall time ~= max-core NEFF exec + dispatch overhead.
    """
    import time
    import jax
    import jax.numpy as jnp
    from jax.sharding import Mesh, PartitionSpec, NamedSharding
    from jax.experimental.shard_map import shard_map
    from concourse import bass2jax, mybir

    bass2jax.install_neuronx_cc_hook()
    nc = _get_nc()

    partition_name = nc.partition_id_tensor.name if nc.partition_id_tensor else None
    in_names, out_names, out_avals = [], [], []
    for alloc in nc.m.functions[0].allocations:
        if not isinstance(alloc, mybir.MemoryLocationSet):
            continue
        name = alloc.memorylocations[0].name
        if alloc.kind == "ExternalInput":
            if name != partition_name:
                in_names.append(name)
        elif alloc.kind == "ExternalOutput":
            out_names.append(name)
            out_avals.append(
                jax.core.ShapedArray(tuple(alloc.tensor_shape), mybir.dt.np(alloc.dtype))
            )
    all_in_names = in_names + out_names
    if partition_name is not None:
        all_in_names = all_in_names + [partition_name]

    def _body(*operands):
        operands = list(operands)
        if partition_name is not None:
            operands.append(bass2jax.partition_id_tensor())
        outs = bass2jax._bass_exec_p.bind(
            *operands,
            out_avals=tuple(out_avals),
            in_names=tuple(all_in_names),
            out_names=tuple(out_names),
            lowering_input_output_aliases=(),
            sim_require_finite=True,
            sim_require_nnan=True,
            nc=nc,
        )
        return tuple(outs)

    devices = jax.devices()[:NCORES]
    mesh = Mesh(np.asarray(devices), ("core",))
    spec = PartitionSpec("core")
    maps = _in_maps(inputs)
    concat = [
        np.concatenate([maps[c][n] for c in range(NCORES)], axis=0) for n in in_names
    ]
    concat += [
        np.zeros((NCORES * a.shape[0], *a.shape[1:]), a.dtype) for a in out_avals
    ]
    sharding = NamedSharding(mesh, spec)
    dev_in = [jax.device_put(a, sharding) for a in concat]

    fn = jax.jit(
        shard_map(
            _body,
            mesh=mesh,
            in_specs=(spec,) * len(concat),
            out_specs=(spec,) * len(out_names),
            check_rep=False,
        )
    )

    for _ in range(warmup):
        outs = fn(*dev_in)
    jax.block_until_ready(outs)

    times = []
    for _ in range(iters):
        t0 = time.perf_counter()
        outs = fn(*dev_in)
        jax.block_until_ready(outs)
        times.append(time.perf_counter() - t0)

    oidx = out_names.index("out")
    o = np.asarray(outs[oidx]).reshape(NCORES, BL, C).reshape(B, C)
    return o.reshape(B, 1, 1, C).astype(np.float32), times
